# revision 7
# baseline (speedup 1.0000x reference)
"""AdaptNet (adaptive-depth MLP cascade) Trainium2 kernel.

Reference semantics: 100 tiny MLP blocks (1->10->10->1, relu) + 100 tiny
classifier MLPs; a while loop runs block c, accumulates
sigmoid(classifier_c(block_out_c)) into `conf` and stops when conf >= 1.0
(or after 100 blocks), returning the last block output.

Key structural fact: block c's output depends only on x (not on block c-1),
so every block + classifier is independent. We compute all 100 in parallel
(blocks mapped to SBUF partitions, 13 per core across 8 cores), then the
early-exit is a host-side cumsum + first-threshold-crossing select over the
gathered per-block (block_out_pre_relu, logit) pairs.

Device program: raw Bass, one serial DVE chain of 8 ops (one DMA in, one
out). Three custom fused DVE micro-ops (registered via the documented
dve_ops extension mechanism, rows 20-22) do most of the work:
  SCAN_RELU_MUL      prefix[e] = cumsum(W[e] * relu(act[e]))  (layer 2,
                     per-neuron sums recovered as strided prefix diffs)
  RELU_MUL_SUM       accum = sum relu(act)*W                  (layer 3)
  MUL_RELUSCALAR_ADD out = W*relu(scalar) + b                 (classifier
                     layer 1 consuming the pre-relu block output)
Relu+bias are folded via an append-1 extended activation vector; the ones
and the prefix-scratch zero column ride in the packed input DMA. The block
output relu is applied on the host (exact in f32).

Packed row layout ([13, 438] f32 per core, one row per block):
  0:10    w1        10:20   b1
  20:130  W2be      k-major [W2[k,0:10], b2[k]] per k
  130:141 W3be      [W3, b3]
  141:151 c1        151:161 d1
  161:271 C2be      271:282 C3be
  282:283 x (replicated)
  283:294 t1e  zeros, col 293 = 1     294:305 t2e  zeros, col 304 = 1
  305:316 u1e  zeros, col 315 = 1     316:327 u2e  zeros, col 326 = 1
  327:438 prefix scratch; col 327 stays 0 (k=0 segment start)
"""

import numpy as np

H = 10
H1 = 11          # extended width (append-1 bias trick)
D = 100          # MAX_DEPTH
NCORES = 8
P = 13           # blocks per core (8*13 = 104, last 4 rows are zero padding)
COLS = 438

_CACHE = {}


def _register_ops():
    """Register the three fused ops in dve_ops.OPS (documented extension
    point; rows 17..31 are free per dve_ops.free_opcode_rows)."""
    if "ops" in _CACHE:
        return _CACHE["ops"]
    import concourse.dve_ops as dve_ops
    from concourse.dve_ops import DveOp, _SUB_OPCODE_FOR_NAME
    from concourse.dve_spec import (
        Spec, Src0, Src1, C0, AluOp, relu, scan, lower, _has_src1)
    from concourse.dve_uop import DveOpSpec

    def make(name, row, spec):
        _SUB_OPCODE_FOR_NAME.setdefault(name, row)
        shas = {}
        for ver in ("v3", "v4"):
            s = DveOpSpec(name=name, opcode=row, uops=lower(spec, ver=ver),
                          rd1_en=_has_src1(spec))
            shas[ver] = s.sha(ver)
        op = DveOp(name, spec, subdim=False, uops_sha=shas)
        if all(o.name != name for o in dve_ops.OPS):
            dve_ops.OPS.append(op)
            dve_ops.CUSTOM_DVE_SPECS[name] = spec
        return op

    def _ref_scan_relu_mul(in0, in1, c0, c1, c2):
        f = lambda a: a.reshape(a.shape[0], -1).astype(np.float32)
        return np.cumsum(f(in0) * np.maximum(f(in1), 0), axis=1,
                         dtype=np.float32).reshape(in0.shape)

    def _ref_relu_mul_sum(in0, in1, c0, c1, c2):
        b = (np.maximum(in0.astype(np.float32), 0) * in1).astype(np.float32)
        return b, b.reshape(b.shape[0], -1).sum(-1, keepdims=True).astype(np.float32)

    def _ref_mul_reluscalar_add(in0, in1, c0, c1, c2):
        return (in0.astype(np.float32) * np.maximum(c0, 0) + in1).astype(np.float32)

    _CACHE["ops"] = (
        make("SCAN_RELU_MUL", 20,
             Spec(body=scan(AluOp.ADD, Src0 * relu(Src1)),
                  reference=_ref_scan_relu_mul)),
        make("RELU_MUL_SUM", 21,
             Spec(body=relu(Src0) * Src1, accum=AluOp.ADD,
                  reference=_ref_relu_mul_sum)),
        make("MUL_RELUSCALAR_ADD", 22,
             Spec(body=Src0 * relu(C0) + Src1,
                  reference=_ref_mul_reluscalar_add)),
    )
    return _CACHE["ops"]


def _build_nc():
    import concourse.bacc as bacc
    import concourse.mybir as mybir

    SCAN_RELU_MUL, RELU_MUL_SUM, MUL_RELUSCALAR_ADD = _register_ops()

    class FastBacc(bacc.Bacc):
        """Bacc that can suppress the init-time const-AP barrier and the
        Block-exit all-engine barrier: the kernel uses no const APs and
        manages its own semaphore lifecycle; NEFF re-execution is already
        serialized by engine halt."""
        _suppress = False

        def all_engine_barrier(self, **kw):
            if self._suppress:
                return
            return super().all_engine_barrier(**kw)

    f32 = mybir.dt.float32
    op = mybir.AluOpType

    FastBacc._suppress = True
    nc = FastBacc("TRN2", target_bir_lowering=False, debug=False)
    FastBacc._suppress = False

    packed = nc.declare_dram_parameter("packed", [P, COLS], f32, isOutput=False)
    out = nc.declare_dram_parameter("out", [P, 2], f32, isOutput=True)

    with (
        nc.sbuf_tensor([P, COLS], f32) as W,
        nc.sbuf_tensor([P, H1], f32) as junk,
        nc.sbuf_tensor([P, 2], f32) as res,
        nc.semaphore() as isem,
        nc.semaphore() as csem,
        nc.semaphore() as osem,
    ):
        try:
            block_cm = nc.Block()
            block = block_cm.__enter__()

            @block.sync
            def _(sync):
                sync.dma_start(W[:, :], packed.ap()).then_inc(isem, 16)
                sync.wait_ge(csem, 1)
                sync.dma_start(out.ap(), res[:, :]).then_inc(osem, 16)
                # semaphore hygiene for NEFF re-execution (profiling loops):
                # cleared after their last consumer; osem is never waited on
                # (the out-DMA lands long before the host reads DRAM) and is
                # cleared on the NEXT run's pass through here.
                sync.sem_clear(isem)
                sync.sem_clear(csem)
                sync.sem_clear(osem)

            @block.vector
            def _(v):
                pfx = W[:, 328:438].rearrange("p (k j) -> p k j", j=H1)
                ends = W[:, 338:438:11]     # prefix at j=10 of each segment k
                starts = W[:, 327:437:11]   # prior segment end (col 327 = 0)

                v.wait_ge(isem, 16)
                # t1 = w1*x + b1
                v.scalar_tensor_tensor(
                    out=W[:, 283:293], in0=W[:, 0:10], scalar=W[:, 282:283],
                    in1=W[:, 10:20], op0=op.mult, op1=op.add)
                v.drain()
                # prefix1 = cumsum(W2be * relu(t1e))
                v._custom_dve(
                    SCAN_RELU_MUL, out=pfx,
                    in0=W[:, 20:130].rearrange("p (k j) -> p k j", j=H1),
                    in1=W[:, 283:294].unsqueeze(1).broadcast_to([P, H, H1]))
                v.drain()
                # t2[k] = prefix[k,10] - prefix[k-1,10]  (= W2 @ relu(t1) + b2)
                v.tensor_tensor(out=W[:, 294:304], in0=ends, in1=starts,
                                op=op.subtract)
                v.drain()
                # o = sum relu(t2e)*W3be -> res[:,0:1]  (host applies relu)
                v._custom_dve(
                    RELU_MUL_SUM, out=junk[:, :], accum_out=res[:, 0:1],
                    in0=W[:, 294:305], in1=W[:, 130:141])
                v.drain()
                # u1 = c1*relu(o) + d1
                v._custom_dve(
                    MUL_RELUSCALAR_ADD, out=W[:, 305:315],
                    in0=W[:, 141:151], in1=W[:, 151:161], s0=res[:, 0:1])
                v.drain()
                # prefix2 = cumsum(C2be * relu(u1e))
                v._custom_dve(
                    SCAN_RELU_MUL, out=pfx,
                    in0=W[:, 161:271].rearrange("p (k j) -> p k j", j=H1),
                    in1=W[:, 305:316].unsqueeze(1).broadcast_to([P, H, H1]))
                v.drain()
                v.tensor_tensor(out=W[:, 316:326], in0=ends, in1=starts,
                                op=op.subtract)
                v.drain()
                # z = sum relu(u2e)*C3be -> res[:,1:2]  (pre-sigmoid logit)
                v._custom_dve(
                    RELU_MUL_SUM, out=junk[:, :], accum_out=res[:, 1:2],
                    in0=W[:, 316:327], in1=W[:, 271:282]).then_inc(csem, 1)

            FastBacc._suppress = True
            block_cm.__exit__(None, None, None)
        finally:
            FastBacc._suppress = False
            nc.cur_block = None

    nc.compile()
    return nc


def _pack(x, Wb1, bb1, Wb2, bb2, Wb3, bb3, Wc1, bc1, Wc2, bc2, Wc3, bc3):
    xs = np.full((D, 1), np.float32(x[0, 0]), dtype=np.float32)
    ext = np.zeros((D, H1), dtype=np.float32)
    ext[:, H] = 1.0
    rows = np.concatenate([
        Wb1[:, :, 0], bb1,
        np.concatenate([Wb2, bb2[:, :, None]], axis=2).reshape(D, H * H1),
        np.concatenate([Wb3[:, 0, :], bb3], axis=1),
        Wc1[:, :, 0], bc1,
        np.concatenate([Wc2, bc2[:, :, None]], axis=2).reshape(D, H * H1),
        np.concatenate([Wc3[:, 0, :], bc3], axis=1),
        xs, ext, ext, ext, ext,
        np.zeros((D, 111), dtype=np.float32),
    ], axis=1).astype(np.float32)
    full = np.zeros((NCORES * P, COLS), dtype=np.float32)
    full[:D] = rows
    # keep the ones columns valid in the padding rows too
    full[D:, [293, 304, 315, 326]] = 1.0
    return full.reshape(NCORES, P, COLS)


def kernel(**inputs):
    from concourse.bass_utils import run_bass_kernel_spmd

    inputs = {k: np.asarray(v, dtype=np.float32) for k, v in inputs.items()}
    if "nc" not in _CACHE:
        _CACHE["nc"] = _build_nc()
    nc = _CACHE["nc"]

    shards = _pack(**inputs)
    in_maps = [{"packed": np.ascontiguousarray(shards[i])} for i in range(NCORES)]
    res = run_bass_kernel_spmd(nc, in_maps, core_ids=list(range(NCORES)))
    outs = np.concatenate([np.asarray(res.results[i]["out"]) for i in range(NCORES)],
                          axis=0)[:D]
    out_c = np.maximum(outs[:, 0], np.float32(0.0))   # block-output relu
    z_c = outs[:, 1]

    # sigmoid via jax CPU so it matches the reference's XLA logistic bit-for-bit
    import jax
    s = np.asarray(
        jax.jit(jax.nn.sigmoid, backend="cpu")(z_c.astype(np.float32)),
        dtype=np.float32)
    cum = np.cumsum(s, dtype=np.float32)
    hit = np.nonzero(cum >= np.float32(1.0))[0]
    idx = int(hit[0]) if hit.size else D - 1
    return np.array([[out_c[idx]]], dtype=np.float32)


# revision 10
# speedup vs baseline: 1.0509x; 1.0509x over previous
"""AdaptNet (adaptive-depth MLP cascade) Trainium2 kernel.

Reference semantics: 100 tiny MLP blocks (1->10->10->1, relu) + 100 tiny
classifier MLPs; a while loop runs block c, accumulates
sigmoid(classifier_c(block_out_c)) into `conf` and stops when conf >= 1.0
(or after 100 blocks), returning the last block output.

Key structural fact: block c's output depends only on x (not on block c-1),
so every block + classifier is independent. We compute all 100 in parallel
(blocks mapped to SBUF partitions, 13 per core across 8 cores), then the
early-exit is a host-side cumsum + first-threshold-crossing select over the
gathered per-block (block_out_pre_relu, logit) pairs.

Device program: raw Bass, one serial DVE chain of 6 ops (one DMA in, one
out). Three custom fused DVE micro-ops (documented dve_ops extension
mechanism, rows 21-24) do the work:
  SEGSCAN_RELU_MUL   per-segment cumsum(W[k,j] * relu(act[j])) - a
                     hand-built FSM (segops-style state surgery) that
                     resets the scan accumulator at each subdim boundary,
                     so out[:,k,10] is exactly layer-2 neuron k's sum
  RELU_MUL_SUM_SEED  accum = s0 + sum relu(act)*W             (layer 3,
                     bias seeded via accum_init=C0)
  MUL_RELUSCALAR_ADD out = W*relu(scalar) + b                 (classifier
                     layer 1 consuming the pre-relu block output)
Layer-2 bias folds via an append-1 extended activation vector (ones ride
in the packed DMA). The block output relu is applied on the host (exact
in f32).

Packed row layout ([13, 438] f32 per core, one row per block):
  0:10    w1        10:20   b1
  20:130  W2be      k-major [W2[k,0:10], b2[k]] per k
  130:141 W3be      [W3, b3]
  141:151 c1        151:161 d1
  161:271 C2be      271:282 C3be
  282:283 x (replicated)
  283:294 t1e  zeros, col 293 = 1     294:305 t2e  zeros, col 304 = 1
  305:316 u1e  zeros, col 315 = 1     316:327 u2e  zeros, col 326 = 1
  327:438 prefix scratch; col 327 stays 0 (k=0 segment start)
"""

import dataclasses

import numpy as np

H = 10
H1 = 11          # extended width (append-1 bias trick)
D = 100          # MAX_DEPTH
NCORES = 8
P = 13           # blocks per core (8*13 = 104, last 4 rows are zero padding)
COLS = 438

_CACHE = {}


def make_segscan_op(row: int):
    import concourse.dve_ops as dve_ops
    from concourse.dve_ops import DveOp, _SUB_OPCODE_FOR_NAME
    from concourse import dve_spec as ds
    from concourse.dve_spec import (
        Spec, Src0, Src1, AluOp, relu, scan, Zero, Trigger, _has_src1)
    from concourse.dve_uop import DveOpSpec

    name = "SEGSCAN_RELU_MUL"

    def _ref(in0, in1, c0, c1, c2):
        a = in0.astype(np.float32)
        b = np.maximum(in1.astype(np.float32), 0)
        return np.cumsum(a * b, axis=-1, dtype=np.float32)

    spec = Spec(body=scan(AluOp.ADD, Src0 * relu(Src1)), reference=_ref)

    def _lower_seg(ver):
        n_lanes, n_stages = ds.N_LANES[ver], ds.N_STAGES[ver]
        ds._validate_body(spec, ver)
        sp = ds._hoist_stream_invariant_ops(spec)
        scans = ds._collect(sp.body, ds.Scan)
        latches = ds._collect(sp.body, ds.Latch)
        p = ds._build_placement(sp, scans, n_stages, n_lanes)
        states = list(ds._build_state_machine(sp, scans, latches, p))
        # last state is the steady; rebuild it as (steady, step)
        steady = states.pop()
        assert steady.trigger == ds.SRC_DONE and not steady.overrides
        steady_idx = len(states)
        step_idx = steady_idx + 1
        sc = scans[0]
        d = p.node_stage[sc]
        step_ov = {d: ds._Stage(sc.op, Zero, sc.expr)}
        states.append(dataclasses.replace(
            steady,
            trigger=(Trigger.SRC_TENSOR_DONE, Trigger.SUB_DIM_DONE,
                     Trigger.NONE),
            next=(0, step_idx, 0)))
        states.append(dataclasses.replace(
            steady, overrides=step_ov,
            trigger=(Trigger.SRC_TENSOR_DONE, Trigger.SUB_DIM_DONE,
                     Trigger.COUNT),
            next=(0, step_idx, steady_idx), repeat=1))
        uops = [ds._assemble(s) for s in states]
        for u in uops:
            u.validate(ver)
        return DveOpSpec(name=name, opcode=row, uops=uops,
                         rd1_en=_has_src1(spec))

    compiled = {ver: _lower_seg(ver) for ver in ("v3", "v4")}

    class SegDveOp(DveOp):
        def compile(self, ver):
            return compiled[ver]

    _SUB_OPCODE_FOR_NAME.setdefault(name, row)
    op = SegDveOp(name, spec, subdim=True,
                  uops_sha={v: s.sha(v) for v, s in compiled.items()})
    if all(o.name != name for o in dve_ops.OPS):
        dve_ops.OPS.append(op)
        dve_ops.CUSTOM_DVE_SPECS[name] = spec
    return op


def _register_ops():
    """Register the three fused ops in dve_ops.OPS (documented extension
    point; rows 17..31 are free per dve_ops.free_opcode_rows)."""
    if "ops" in _CACHE:
        return _CACHE["ops"]
    import concourse.dve_ops as dve_ops
    from concourse.dve_ops import DveOp, _SUB_OPCODE_FOR_NAME
    from concourse.dve_spec import (
        Spec, Src0, Src1, C0, AluOp, relu, scan, lower, _has_src1)
    from concourse.dve_uop import DveOpSpec

    def make(name, row, spec):
        _SUB_OPCODE_FOR_NAME.setdefault(name, row)
        shas = {}
        for ver in ("v3", "v4"):
            s = DveOpSpec(name=name, opcode=row, uops=lower(spec, ver=ver),
                          rd1_en=_has_src1(spec))
            shas[ver] = s.sha(ver)
        op = DveOp(name, spec, subdim=False, uops_sha=shas)
        if all(o.name != name for o in dve_ops.OPS):
            dve_ops.OPS.append(op)
            dve_ops.CUSTOM_DVE_SPECS[name] = spec
        return op

    def _ref_mul_reluscalar_add(in0, in1, c0, c1, c2):
        return (in0.astype(np.float32) * np.maximum(c0, 0) + in1).astype(np.float32)

    def _ref_relu_mul_sum_seed(in0, in1, c0, c1, c2):
        b = (np.maximum(in0.astype(np.float32), 0) * in1).astype(np.float32)
        s = (c0 + b.reshape(b.shape[0], -1).sum(-1, keepdims=True)
             ).astype(np.float32)
        return b, s

    _CACHE["ops"] = (
        make_segscan_op(23),
        make("RELU_MUL_SUM_SEED", 24,
             Spec(body=relu(Src0) * Src1, accum=AluOp.ADD, accum_init=C0,
                  reference=_ref_relu_mul_sum_seed)),
        make("MUL_RELUSCALAR_ADD", 22,
             Spec(body=Src0 * relu(C0) + Src1,
                  reference=_ref_mul_reluscalar_add)),
    )
    return _CACHE["ops"]


def _build_nc():
    import concourse.bacc as bacc
    import concourse.mybir as mybir

    SEGSCAN_RELU_MUL, RELU_MUL_SUM_SEED, MUL_RELUSCALAR_ADD = _register_ops()

    class FastBacc(bacc.Bacc):
        """Bacc that can suppress the init-time const-AP barrier and the
        Block-exit all-engine barrier: the kernel uses no const APs and
        manages its own semaphore lifecycle; NEFF re-execution is already
        serialized by engine halt."""
        _suppress = False

        def all_engine_barrier(self, **kw):
            if self._suppress:
                return
            return super().all_engine_barrier(**kw)

    f32 = mybir.dt.float32
    op = mybir.AluOpType

    FastBacc._suppress = True
    nc = FastBacc("TRN2", target_bir_lowering=False, debug=False)
    FastBacc._suppress = False

    packed = nc.declare_dram_parameter("packed", [P, COLS], f32, isOutput=False)
    out = nc.declare_dram_parameter("out", [P, 2], f32, isOutput=True)

    with (
        nc.sbuf_tensor([P, COLS], f32) as W,
        nc.sbuf_tensor([P, H], f32) as junk,
        nc.sbuf_tensor([P, 2], f32) as res,
        nc.semaphore() as isem,
        nc.semaphore() as csem,
        nc.semaphore() as osem,
    ):
        try:
            block_cm = nc.Block()
            block = block_cm.__enter__()

            @block.sync
            def _(sync):
                sync.dma_start(W[:, :], packed.ap()).then_inc(isem, 16)
                sync.wait_ge(csem, 1)
                sync.dma_start(out.ap(), res[:, :]).then_inc(osem, 16)
                # semaphore hygiene for NEFF re-execution (profiling loops):
                # cleared after their last consumer; osem is never waited on
                # (the out-DMA lands long before the host reads DRAM) and is
                # cleared on the NEXT run's pass through here.
                sync.sem_clear(isem)
                sync.sem_clear(csem)
                sync.sem_clear(osem)

            @block.vector
            def _(v):
                pfx = W[:, 328:438].rearrange("p (k j) -> p k j", j=H1)
                ends = W[:, 338:438:11]     # segment sums (prefix at j=10)

                v.wait_ge(isem, 16)
                # t1 = w1*x + b1
                v.scalar_tensor_tensor(
                    out=W[:, 283:293], in0=W[:, 0:10], scalar=W[:, 282:283],
                    in1=W[:, 10:20], op0=op.mult, op1=op.add)
                v.drain()
                # seg-prefix1: pfx[:,k,10] = sum_j W2be[k,j]*relu(t1e[j])
                #            = (W2 @ relu(t1) + b2)[k] = t2[k]
                v._custom_dve(
                    SEGSCAN_RELU_MUL, out=pfx,
                    in0=W[:, 20:130].rearrange("p (k j) -> p k j", j=H1),
                    in1=W[:, 283:294].unsqueeze(1).broadcast_to([P, H, H1]))
                v.drain()
                # o = b3 + sum relu(t2)*W3 -> res[:,0:1] (host applies relu)
                v._custom_dve(
                    RELU_MUL_SUM_SEED, out=junk[:, :], accum_out=res[:, 0:1],
                    in0=ends, in1=W[:, 130:140], s0=W[:, 140:141])
                v.drain()
                # u1 = c1*relu(o) + d1
                v._custom_dve(
                    MUL_RELUSCALAR_ADD, out=W[:, 305:315],
                    in0=W[:, 141:151], in1=W[:, 151:161], s0=res[:, 0:1])
                v.drain()
                # seg-prefix2: pfx[:,k,10] = u2[k]
                v._custom_dve(
                    SEGSCAN_RELU_MUL, out=pfx,
                    in0=W[:, 161:271].rearrange("p (k j) -> p k j", j=H1),
                    in1=W[:, 305:316].unsqueeze(1).broadcast_to([P, H, H1]))
                v.drain()
                # z = bc3 + sum relu(u2)*C3 -> res[:,1:2] (pre-sigmoid logit)
                v._custom_dve(
                    RELU_MUL_SUM_SEED, out=junk[:, :], accum_out=res[:, 1:2],
                    in0=ends, in1=W[:, 271:281],
                    s0=W[:, 281:282]).then_inc(csem, 1)

            FastBacc._suppress = True
            block_cm.__exit__(None, None, None)
        finally:
            FastBacc._suppress = False
            nc.cur_block = None

    nc.compile()
    return nc


def _pack(x, Wb1, bb1, Wb2, bb2, Wb3, bb3, Wc1, bc1, Wc2, bc2, Wc3, bc3):
    xs = np.full((D, 1), np.float32(x[0, 0]), dtype=np.float32)
    ext = np.zeros((D, H1), dtype=np.float32)
    ext[:, H] = 1.0
    rows = np.concatenate([
        Wb1[:, :, 0], bb1,
        np.concatenate([Wb2, bb2[:, :, None]], axis=2).reshape(D, H * H1),
        np.concatenate([Wb3[:, 0, :], bb3], axis=1),
        Wc1[:, :, 0], bc1,
        np.concatenate([Wc2, bc2[:, :, None]], axis=2).reshape(D, H * H1),
        np.concatenate([Wc3[:, 0, :], bc3], axis=1),
        xs, ext, ext, ext, ext,
        np.zeros((D, 111), dtype=np.float32),
    ], axis=1).astype(np.float32)
    full = np.zeros((NCORES * P, COLS), dtype=np.float32)
    full[:D] = rows
    # keep the ones columns valid in the padding rows too
    full[D:, [293, 304, 315, 326]] = 1.0
    return full.reshape(NCORES, P, COLS)


def kernel(**inputs):
    from concourse.bass_utils import run_bass_kernel_spmd

    inputs = {k: np.asarray(v, dtype=np.float32) for k, v in inputs.items()}
    if "nc" not in _CACHE:
        _CACHE["nc"] = _build_nc()
    nc = _CACHE["nc"]

    shards = _pack(**inputs)
    in_maps = [{"packed": np.ascontiguousarray(shards[i])} for i in range(NCORES)]
    res = run_bass_kernel_spmd(nc, in_maps, core_ids=list(range(NCORES)))
    outs = np.concatenate([np.asarray(res.results[i]["out"]) for i in range(NCORES)],
                          axis=0)[:D]
    out_c = np.maximum(outs[:, 0], np.float32(0.0))   # block-output relu
    z_c = outs[:, 1]

    # sigmoid via jax CPU so it matches the reference's XLA logistic bit-for-bit
    import jax
    s = np.asarray(
        jax.jit(jax.nn.sigmoid, backend="cpu")(z_c.astype(np.float32)),
        dtype=np.float32)
    cum = np.cumsum(s, dtype=np.float32)
    hit = np.nonzero(cum >= np.float32(1.0))[0]
    idx = int(hit[0]) if hit.size else D - 1
    return np.array([[out_c[idx]]], dtype=np.float32)


# revision 11
# speedup vs baseline: 1.0638x; 1.0122x over previous
"""AdaptNet (adaptive-depth MLP cascade) Trainium2 kernel.

Reference semantics: 100 tiny MLP blocks (1->10->10->1, relu) + 100 tiny
classifier MLPs; a while loop runs block c, accumulates
sigmoid(classifier_c(block_out_c)) into `conf` and stops when conf >= 1.0
(or after 100 blocks), returning the last block output.

Key structural fact: block c's output depends only on x (not on block c-1),
so every block + classifier is independent. We compute all 100 in parallel
(blocks mapped to SBUF partitions, 13 per core across 8 cores), then the
early-exit is a host-side cumsum + first-threshold-crossing select over the
gathered per-block (block_out_pre_relu, logit) pairs.

Device program: raw Bass, one serial DVE chain of 6 ops (one DMA in, one
out). Three custom fused DVE micro-ops (documented dve_ops extension
mechanism, rows 21-24) do the work:
  SEGSCAN_RELU_MUL   per-segment cumsum(W[k,j] * relu(act[j])) - a
                     hand-built FSM (segops-style state surgery) that
                     resets the scan accumulator at each subdim boundary,
                     so out[:,k,10] is exactly layer-2 neuron k's sum
  RELU_MUL_SUM_SEED  accum = s0 + sum relu(act)*W             (layer 3,
                     bias seeded via accum_init=C0)
  MUL_RELUSCALAR_ADD out = W*relu(scalar) + b                 (classifier
                     layer 1 consuming the pre-relu block output)
Layer-2 bias folds via an append-1 extended activation vector (ones ride
in the packed DMA). The block output relu is applied on the host (exact
in f32).

Packed row layout ([13, 438] f32 per core, one row per block):
  0:10    w1        10:20   b1
  20:130  W2be      k-major [W2[k,0:10], b2[k]] per k
  130:141 W3be      [W3, b3]
  141:151 c1        151:161 d1
  161:271 C2be      271:282 C3be
  282:283 x (replicated)
  283:294 t1e  zeros, col 293 = 1     294:305 t2e  zeros, col 304 = 1
  305:316 u1e  zeros, col 315 = 1     316:327 u2e  zeros, col 326 = 1
  327:438 prefix scratch; col 327 stays 0 (k=0 segment start)
"""

import dataclasses

import numpy as np

H = 10
H1 = 11          # extended width (append-1 bias trick)
D = 100          # MAX_DEPTH
NCORES = 8
P = 13           # blocks per core (8*13 = 104, last 4 rows are zero padding)
COLS = 438

_CACHE = {}


def make_segscan_op(row: int):
    import concourse.dve_ops as dve_ops
    from concourse.dve_ops import DveOp, _SUB_OPCODE_FOR_NAME
    from concourse import dve_spec as ds
    from concourse.dve_spec import (
        Spec, Src0, Src1, AluOp, relu, scan, Zero, Trigger, _has_src1)
    from concourse.dve_uop import DveOpSpec

    name = "SEGSCAN_RELU_MUL"

    def _ref(in0, in1, c0, c1, c2):
        a = in0.astype(np.float32)
        b = np.maximum(in1.astype(np.float32), 0)
        return np.cumsum(a * b, axis=-1, dtype=np.float32)

    spec = Spec(body=scan(AluOp.ADD, Src0 * relu(Src1)), reference=_ref)

    def _lower_seg(ver):
        n_lanes, n_stages = ds.N_LANES[ver], ds.N_STAGES[ver]
        ds._validate_body(spec, ver)
        sp = ds._hoist_stream_invariant_ops(spec)
        scans = ds._collect(sp.body, ds.Scan)
        latches = ds._collect(sp.body, ds.Latch)
        p = ds._build_placement(sp, scans, n_stages, n_lanes)
        states = list(ds._build_state_machine(sp, scans, latches, p))
        # last state is the steady; rebuild it as (steady, step)
        steady = states.pop()
        assert steady.trigger == ds.SRC_DONE and not steady.overrides
        steady_idx = len(states)
        step_idx = steady_idx + 1
        sc = scans[0]
        d = p.node_stage[sc]
        step_ov = {d: ds._Stage(sc.op, Zero, sc.expr)}
        states.append(dataclasses.replace(
            steady,
            trigger=(Trigger.SRC_TENSOR_DONE, Trigger.SUB_DIM_DONE,
                     Trigger.NONE),
            next=(0, step_idx, 0)))
        states.append(dataclasses.replace(
            steady, overrides=step_ov,
            trigger=(Trigger.SRC_TENSOR_DONE, Trigger.SUB_DIM_DONE,
                     Trigger.COUNT),
            next=(0, step_idx, steady_idx), repeat=1))
        uops = [ds._assemble(s) for s in states]
        for u in uops:
            u.validate(ver)
        return DveOpSpec(name=name, opcode=row, uops=uops,
                         rd1_en=_has_src1(spec))

    compiled = {ver: _lower_seg(ver) for ver in ("v3", "v4")}

    class SegDveOp(DveOp):
        def compile(self, ver):
            return compiled[ver]

    _SUB_OPCODE_FOR_NAME.setdefault(name, row)
    op = SegDveOp(name, spec, subdim=True,
                  uops_sha={v: s.sha(v) for v, s in compiled.items()})
    if all(o.name != name for o in dve_ops.OPS):
        dve_ops.OPS.append(op)
        dve_ops.CUSTOM_DVE_SPECS[name] = spec
    return op


def _register_ops():
    """Register the three fused ops in dve_ops.OPS (documented extension
    point; rows 17..31 are free per dve_ops.free_opcode_rows)."""
    if "ops" in _CACHE:
        return _CACHE["ops"]
    import concourse.dve_ops as dve_ops
    from concourse.dve_ops import DveOp, _SUB_OPCODE_FOR_NAME
    from concourse.dve_spec import (
        Spec, Src0, Src1, C0, AluOp, relu, scan, lower, _has_src1)
    from concourse.dve_uop import DveOpSpec

    def make(name, row, spec):
        _SUB_OPCODE_FOR_NAME.setdefault(name, row)
        shas = {}
        for ver in ("v3", "v4"):
            s = DveOpSpec(name=name, opcode=row, uops=lower(spec, ver=ver),
                          rd1_en=_has_src1(spec))
            shas[ver] = s.sha(ver)
        op = DveOp(name, spec, subdim=False, uops_sha=shas)
        if all(o.name != name for o in dve_ops.OPS):
            dve_ops.OPS.append(op)
            dve_ops.CUSTOM_DVE_SPECS[name] = spec
        return op

    def _ref_mul_reluscalar_add(in0, in1, c0, c1, c2):
        return (in0.astype(np.float32) * np.maximum(c0, 0) + in1).astype(np.float32)

    def _ref_relu_mul_sum_seed(in0, in1, c0, c1, c2):
        b = (np.maximum(in0.astype(np.float32), 0) * in1).astype(np.float32)
        s = (c0 + b.reshape(b.shape[0], -1).sum(-1, keepdims=True)
             ).astype(np.float32)
        return b, s

    _CACHE["ops"] = (
        make_segscan_op(23),
        make("RELU_MUL_SUM_SEED", 24,
             Spec(body=relu(Src0) * Src1, accum=AluOp.ADD, accum_init=C0,
                  reference=_ref_relu_mul_sum_seed)),
        make("MUL_RELUSCALAR_ADD", 22,
             Spec(body=Src0 * relu(C0) + Src1,
                  reference=_ref_mul_reluscalar_add)),
    )
    return _CACHE["ops"]


def _build_nc():
    import concourse.bacc as bacc
    import concourse.mybir as mybir

    SEGSCAN_RELU_MUL, RELU_MUL_SUM_SEED, MUL_RELUSCALAR_ADD = _register_ops()

    class FastBacc(bacc.Bacc):
        """Bacc that can suppress the init-time const-AP barrier and the
        Block-exit all-engine barrier: the kernel uses no const APs and
        manages its own semaphore lifecycle; NEFF re-execution is already
        serialized by engine halt."""
        _suppress = False

        def all_engine_barrier(self, **kw):
            if self._suppress:
                return
            return super().all_engine_barrier(**kw)

    f32 = mybir.dt.float32
    op = mybir.AluOpType

    FastBacc._suppress = True
    nc = FastBacc("TRN2", target_bir_lowering=False, debug=False)
    FastBacc._suppress = False

    packed = nc.declare_dram_parameter("packed", [P, COLS], f32, isOutput=False)
    out = nc.declare_dram_parameter("out", [P, 2], f32, isOutput=True)

    with (
        nc.sbuf_tensor([P, COLS], f32) as W,
        nc.sbuf_tensor([P, H], f32) as junk,
        nc.sbuf_tensor([P, 2], f32) as res,
        nc.semaphore() as isem,
        nc.semaphore() as csem,
        nc.semaphore() as osem,
    ):
        # No nc.Block(): per-engine streams are emitted straight into the
        # entry basic block (engine tag orders them), skipping the entry/exit
        # branches on every engine's critical path.
        if True:
            sync = nc.sync
            # DMA only the live columns (316:438 is scratch, written before
            # read); the packed layout is unchanged.
            sync.dma_start(W[:, 0:316], packed.ap()[:, 0:316]).then_inc(isem, 16)
            sync.wait_ge(csem, 1)
            sync.dma_start(out.ap(), res[:, :]).then_inc(osem, 16)
            # csem cleared here (only SP knows its wait passed); isem/osem
            # are cleared on the DVE tail - see below.
            sync.sem_clear(csem)

            if True:
                v = nc.vector
                pfx = W[:, 328:438].rearrange("p (k j) -> p k j", j=H1)
                ends = W[:, 338:438:11]     # segment sums (prefix at j=10)

                v.wait_ge(isem, 16)
                # t1 = w1*x + b1
                v.scalar_tensor_tensor(
                    out=W[:, 283:293], in0=W[:, 0:10], scalar=W[:, 282:283],
                    in1=W[:, 10:20], op0=op.mult, op1=op.add)
                v.drain()
                # seg-prefix1: pfx[:,k,10] = sum_j W2be[k,j]*relu(t1e[j])
                #            = (W2 @ relu(t1) + b2)[k] = t2[k]
                v._custom_dve(
                    SEGSCAN_RELU_MUL, out=pfx,
                    in0=W[:, 20:130].rearrange("p (k j) -> p k j", j=H1),
                    in1=W[:, 283:294].unsqueeze(1).broadcast_to([P, H, H1]))
                v.drain()
                # o = b3 + sum relu(t2)*W3 -> res[:,0:1] (host applies relu)
                v._custom_dve(
                    RELU_MUL_SUM_SEED, out=junk[:, :], accum_out=res[:, 0:1],
                    in0=ends, in1=W[:, 130:140], s0=W[:, 140:141])
                v.drain()
                # u1 = c1*relu(o) + d1
                v._custom_dve(
                    MUL_RELUSCALAR_ADD, out=W[:, 305:315],
                    in0=W[:, 141:151], in1=W[:, 151:161], s0=res[:, 0:1])
                v.drain()
                # seg-prefix2: pfx[:,k,10] = u2[k]
                v._custom_dve(
                    SEGSCAN_RELU_MUL, out=pfx,
                    in0=W[:, 161:271].rearrange("p (k j) -> p k j", j=H1),
                    in1=W[:, 305:316].unsqueeze(1).broadcast_to([P, H, H1]))
                v.drain()
                # z = bc3 + sum relu(u2)*C3 -> res[:,1:2] (pre-sigmoid logit)
                v._custom_dve(
                    RELU_MUL_SUM_SEED, out=junk[:, :], accum_out=res[:, 1:2],
                    in0=ends, in1=W[:, 271:281],
                    s0=W[:, 281:282]).then_inc(csem, 1)
                # semaphore hygiene for NEFF re-execution, off the SP tail:
                # isem was consumed by this engine; osem (never waited) still
                # holds the PREVIOUS run's +16 - this run's inc lands ~2us
                # after this clear, leaving the steady 16 for the next run.
                v.sem_clear(isem)
                v.sem_clear(osem)

    nc.compile()
    return nc


def _pack(x, Wb1, bb1, Wb2, bb2, Wb3, bb3, Wc1, bc1, Wc2, bc2, Wc3, bc3):
    xs = np.full((D, 1), np.float32(x[0, 0]), dtype=np.float32)
    ext = np.zeros((D, H1), dtype=np.float32)
    ext[:, H] = 1.0
    rows = np.concatenate([
        Wb1[:, :, 0], bb1,
        np.concatenate([Wb2, bb2[:, :, None]], axis=2).reshape(D, H * H1),
        np.concatenate([Wb3[:, 0, :], bb3], axis=1),
        Wc1[:, :, 0], bc1,
        np.concatenate([Wc2, bc2[:, :, None]], axis=2).reshape(D, H * H1),
        np.concatenate([Wc3[:, 0, :], bc3], axis=1),
        xs, ext, ext, ext, ext,
        np.zeros((D, 111), dtype=np.float32),
    ], axis=1).astype(np.float32)
    full = np.zeros((NCORES * P, COLS), dtype=np.float32)
    full[:D] = rows
    # keep the ones columns valid in the padding rows too
    full[D:, [293, 304, 315, 326]] = 1.0
    return full.reshape(NCORES, P, COLS)


def kernel(**inputs):
    from concourse.bass_utils import run_bass_kernel_spmd

    inputs = {k: np.asarray(v, dtype=np.float32) for k, v in inputs.items()}
    if "nc" not in _CACHE:
        _CACHE["nc"] = _build_nc()
    nc = _CACHE["nc"]

    shards = _pack(**inputs)
    in_maps = [{"packed": np.ascontiguousarray(shards[i])} for i in range(NCORES)]
    res = run_bass_kernel_spmd(nc, in_maps, core_ids=list(range(NCORES)))
    outs = np.concatenate([np.asarray(res.results[i]["out"]) for i in range(NCORES)],
                          axis=0)[:D]
    out_c = np.maximum(outs[:, 0], np.float32(0.0))   # block-output relu
    z_c = outs[:, 1]

    # sigmoid via jax CPU so it matches the reference's XLA logistic bit-for-bit
    import jax
    s = np.asarray(
        jax.jit(jax.nn.sigmoid, backend="cpu")(z_c.astype(np.float32)),
        dtype=np.float32)
    cum = np.cumsum(s, dtype=np.float32)
    hit = np.nonzero(cum >= np.float32(1.0))[0]
    idx = int(hit[0]) if hit.size else D - 1
    return np.array([[out_c[idx]]], dtype=np.float32)
